# revision 12
# baseline (speedup 1.0000x reference)
"""Trainium2 Bass kernel for DynamicABPINN (moe_routing, dense evaluation).

Model: 8 gated subnets (4 hidden tanh layers of width 64 each), Gaussian-window
softmax gating over subnets, periodic input embedding, hard-constraint output.

Strategy (v2, ACT-bottleneck optimized):
  - Pure data parallel over 8 NeuronCores: each core handles N/8 = 131072 points.
  - All matmul datapaths fp16/f32r (1 cyc/row); hidden state fp16 end to end.
  - Gating exp batched across groups of 4 tiles into one [32, F] psum via
    sparse-column lhsT accumulation; one ACT Exp per group.
  - ACT does 14 of 16 pair-layer tanhs per tile; layer-4 pairs 0/1 are
    offloaded: GPSIMD (Pool) drains psum -> fp16 with scale/8,bias/8, then an
    11-op DVE fp16 rational approximates tanh (max err ~1.5e-3, end-to-end
    rel err ~8e-4 incl. fp16 rounding).
  - Combine: eo on DVE, numer/denom via one fp16 place matmul pair into psum
    partitions 32:34 (tile_position col 32), Pool drains to SBUF, DMA out.
"""

import sys

for _p in ("/opt/trn_rl_repo", "/root/.axon_site/_ro/trn_rl_repo"):
    if _p not in sys.path:
        sys.path.insert(0, _p)

import numpy as np

import concourse.bass as bass
import concourse.bacc as bacc
import concourse.mybir as mybir
from concourse.tile import TileContext
from concourse.tile_rust import add_dep_helper
from concourse.bass_utils import run_bass_kernel_spmd


def _strict_barrier(tc, nc):
    """strict_bb_all_engine_barrier anchored on a DRAIN instruction (walrus
    caps queue instructions at one embedded sem wait, except DRAIN)."""
    curr_bb = nc.cur_bb
    assert curr_bb is not None
    prev_insts = list(curr_bb.bb.instructions)
    barrier_instruction = nc.sync.drain()
    tc.barrier_instruction_and_bb = (barrier_instruction.ins, curr_bb)
    if (
        tc.no_sync_barrier_and_bb is not None
        and tc.no_sync_barrier_and_bb[1] == curr_bb
    ):
        tc.no_sync_barrier_and_bb = None
    for instruction in prev_insts:
        add_dep_helper(
            barrier_instruction.ins,
            instruction,
            sync=bass.sync_unless_reorderable_target(
                instruction, instruction.is_executable()
            ),
            reason="strict_bb_all_engine_barrier: backward edge",
        )

F32 = mybir.dt.float32
F32R = mybir.dt.float32r
F16 = mybir.dt.float16
AF = mybir.ActivationFunctionType
OP = mybir.AluOpType

N = 1048576
NCORES = 8
NC_PTS = N // NCORES          # 131072 points per core
P = 128                       # partitions
NJ = NC_PTS // P              # 1024 point-major columns
NT = P                        # 128 feature-major tiles of F=1024 points
F = NJ                        # 1024 points per tile
CH = 512                      # matmul moving-operand chunk (fp32 max, 1 psum bank)
K = 8                         # subnets
H = 64                        # hidden width
NPAIR = 4                     # subnet pairs packed into 128 partitions
PI = float(np.pi)

# rational tanh(y) ~ clamp(C2*z*(p^2+c1'p+c0')/(p^2+d1'p+d0')), z=y/8, p=z^2
_RC0, _RC1, _RC2 = 1.45060304e+03, 1.49308936e+02, 5.30585751e-02
_RD0, _RD1 = 7.69755550e+01, 3.35529749e+01
RC1 = _RC1 / 64.0
RC0 = _RC0 / 4096.0
RD1 = _RD1 / 64.0
RD0 = _RD0 / 4096.0
RC2 = 8.0 * _RC2

# column offsets inside the packed fp32 constant tensor
_COL_SIZES = (
    ("bsc", 16), ("ssc", 16), ("bs8", 2), ("ss8", 2), ("eb4", 1),
    ("cxn", K), ("ctn", K), ("gxv", K), ("gtv", K), ("trig", 2),
)
COL = {}
_off = 0
for _name, _sz in _COL_SIZES:
    COL[_name] = _off
    _off += _sz
CPACK_W = _off

# float32r pack: 4 slot-variant gating lhsT blocks [5, 32] each
WPACK_W = 128

# fp16 weight pack
_W16_SIZES = (
    ("w1", NPAIR * P), ("w2", NPAIR * P), ("w3", NPAIR * P), ("w4", NPAIR * P),
    ("w5", NPAIR * K), ("selB", 2), ("selO", 1),
)
W16 = {}
_off = 0
for _name, _sz in _W16_SIZES:
    W16[_name] = _off
    _off += _sz
W16_W = _off

_CACHE = {}


def _build_program():
    nc = bacc.Bacc()

    x_in = nc.declare_dram_parameter("x_in", [P, NJ], F32, isOutput=False)
    t_in = nc.declare_dram_parameter("t_in", [P, NJ], F32, isOutput=False)
    cpack = nc.declare_dram_parameter("cpack", [P, CPACK_W], F32, isOutput=False)
    wpack = nc.declare_dram_parameter(
        "wpack", [P, WPACK_W], F32R, isOutput=False
    )
    wpk16 = nc.declare_dram_parameter("wpk16", [P, W16_W], F16, isOutput=False)
    u_out = nc.declare_dram_parameter("u_out", [P, NJ], F32, isOutput=True)

    # Internal DRAM staging: point-major -> feature-major relayout.
    stage_inp = nc.dram_tensor("stage_inp", [NT, 6, F], F16)
    stage_gf = nc.dram_tensor("stage_gf", [NT, 5, F], F32R)
    comb_dram = nc.dram_tensor("comb_dram", [NT, 2, F], F32)

    with TileContext(nc) as tc:
        with (
            tc.tile_pool(name="const", bufs=1) as cpool,
            tc.tile_pool(name="pm", bufs=1) as pm,
            tc.tile_pool(name="ld6", bufs=3) as ld6,
            tc.tile_pool(name="ldg", bufs=3) as ldg,
            tc.tile_pool(name="hpool", bufs=12) as hp,
            tc.tile_pool(name="small", bufs=2) as sm,
            tc.tile_pool(name="epool", bufs=3) as ep,
            tc.tile_pool(name="rat", bufs=2) as rp,
            tc.tile_pool(name="pL", bufs=2, space="PSUM") as pL,
            tc.tile_pool(name="pgo", bufs=2, space="PSUM") as pgo,
        ):
            # ---- constants ----
            cpk = cpool.tile([P, CPACK_W], F32, tag="cpk")
            nc.sync.dma_start(out=cpk[:], in_=cpack[:])
            wpk = cpool.tile([P, WPACK_W], F32R, tag="wpk")
            nc.sync.dma_start(out=wpk[:], in_=wpack[:])
            wk16 = cpool.tile([P, W16_W], F16, tag="wk16")
            nc.sync.dma_start(out=wk16[:], in_=wpk16[:])

            w1s = wk16[0:6, W16["w1"]:W16["w1"] + NPAIR * P]
            w2s = wk16[:, W16["w2"]:W16["w2"] + NPAIR * P]
            w3s = wk16[:, W16["w3"]:W16["w3"] + NPAIR * P]
            w4s = wk16[:, W16["w4"]:W16["w4"] + NPAIR * P]
            w5s = wk16[:, W16["w5"]:W16["w5"] + NPAIR * K]
            selB_k = [wk16[32 * k:32 * k + K, W16["selB"]:W16["selB"] + 2]
                      for k in range(3)]
            selOs = wk16[0:K, W16["selO"]:W16["selO"] + 1]
            bscs = cpk[:, COL["bsc"]:COL["bsc"] + 16]
            sscs = cpk[:, COL["ssc"]:COL["ssc"] + 16]
            bs8s = cpk[:, COL["bs8"]:COL["bs8"] + 2]
            ss8s = cpk[:, COL["ss8"]:COL["ss8"] + 2]
            eb4s = cpk[0:72, COL["eb4"]:COL["eb4"] + 1]
            cxns = cpk[:, COL["cxn"]:COL["cxn"] + K]
            ctns = cpk[:, COL["ctn"]:COL["ctn"] + K]
            gxvs = cpk[:, COL["gxv"]:COL["gxv"] + K]
            gtvs = cpk[:, COL["gtv"]:COL["gtv"] + K]
            trgc = cpk[:, COL["trig"]:COL["trig"] + 2]

            # engine clock warmup (single embedded sem wait per engine)
            wdum = sm.tile([1, 4], F32, tag="wdum")
            nc.scalar.activation(wdum[0:1, 0:1], cpk[0:1, 0:1], AF.Copy)

            # ---- Phase S: point-major prep ----
            tanht = pm.tile([P, NJ], F32, tag="tanht")
            u0 = pm.tile([P, NJ], F32, tag="u0")
            with tc.tile_pool(name="sphase", bufs=1) as sp:
                x_pm = sp.tile([P, NJ], F32, tag="x_pm")
                t_pm = sp.tile([P, NJ], F32, tag="t_pm")
                nc.sync.dma_start(out=x_pm[:], in_=x_in[:])
                nc.sync.dma_start(out=t_pm[:], in_=t_in[:])

                cosx = sp.tile([P, NJ], F32, tag="cosx")
                cosr = sp.tile([P, NJ], F16, tag="cosr")
                sinr = sp.tile([P, NJ], F16, tag="sinr")
                t_r = sp.tile([P, NJ], F16, tag="t_r")
                x2 = sp.tile([P, NJ], F32, tag="x2")
                t2 = sp.tile([P, NJ], F32, tag="t2")
                mneg = sp.tile([P, NJ], F32, tag="mneg")
                lg_all = sp.tile([P, NJ * (K // 2)], F32, tag="lg_all")

                # ACT Sin has no range reduction: reduce args to [-1, 1]
                # periods via fp32 magic-number rounding, r = z - 2*round(z/2).
                MAGIC = float(1.5 * 2 ** 23)
                scr1 = sp.tile([P, NJ], F32, tag="scr1")
                scr2 = sp.tile([P, NJ], F32, tag="scr2")
                # sin(pi x): r = x - 2 round(x/2)
                nc.vector.tensor_scalar(
                    out=scr1[:], in0=x_pm[:], scalar1=0.5, scalar2=MAGIC,
                    op0=OP.mult, op1=OP.add,
                )
                nc.vector.tensor_scalar(
                    out=scr1[:], in0=scr1[:], scalar1=MAGIC, scalar2=-2.0,
                    op0=OP.subtract, op1=OP.mult,
                )
                nc.vector.tensor_tensor(
                    out=scr1[:], in0=x_pm[:], in1=scr1[:], op=OP.add
                )
                nc.scalar.activation(
                    sinr[:], scr1[:], AF.Sin, bias=trgc[:, 1:2], scale=PI
                )
                # cos(pi x) = sin(pi (x + 1/2)): same reduction on y = x + 0.5
                nc.vector.tensor_scalar(
                    out=scr2[:], in0=x_pm[:], scalar1=0.5, scalar2=0.25,
                    op0=OP.mult, op1=OP.add,
                )
                nc.vector.tensor_scalar(
                    out=scr2[:], in0=scr2[:], scalar1=MAGIC, scalar2=MAGIC,
                    op0=OP.add, op1=OP.subtract,
                )
                nc.vector.tensor_scalar(
                    out=scr2[:], in0=scr2[:], scalar1=-2.0, scalar2=0.5,
                    op0=OP.mult, op1=OP.add,
                )
                nc.vector.tensor_tensor(
                    out=scr2[:], in0=x_pm[:], in1=scr2[:], op=OP.add
                )
                nc.scalar.activation(
                    cosx[:], scr2[:], AF.Sin, bias=trgc[:, 1:2], scale=PI
                )
                nc.scalar.activation(
                    cosr[:], scr2[:], AF.Sin, bias=trgc[:, 1:2], scale=PI
                )
                nc.scalar.activation(t_r[:], t_pm[:], AF.Copy)
                nc.scalar.activation(tanht[:], t_pm[:], AF.Tanh, bias=trgc[:, 1:2])
                nc.vector.tensor_tensor(
                    out=x2[:], in0=x_pm[:], in1=x_pm[:], op=OP.mult
                )
                nc.vector.tensor_tensor(
                    out=t2[:], in0=t_pm[:], in1=t_pm[:], op=OP.mult
                )
                tc.no_sync_barrier()

                # gating quadratics, interleaved [P, NJ, K/2] x 2 halves
                lg_v = lg_all[:].rearrange("p (j k) -> p j k", k=K // 2)
                for half in range(2):
                    for ii in range(K // 2):
                        i = half * (K // 2) + ii
                        nc.vector.tensor_scalar_add(
                            scr1[:], x_pm[:], cxns[:, i:i + 1]
                        )
                        nc.vector.tensor_tensor(
                            out=scr2[:], in0=scr1[:], in1=scr1[:], op=OP.mult
                        )
                        nc.vector.tensor_scalar_mul(
                            lg_v[:, :, ii], scr2[:], gxvs[:, i:i + 1]
                        )
                        nc.vector.tensor_scalar_add(
                            scr1[:], t_pm[:], ctns[:, i:i + 1]
                        )
                        nc.vector.tensor_tensor(
                            out=scr2[:], in0=scr1[:], in1=scr1[:], op=OP.mult
                        )
                        nc.vector.tensor_scalar_mul(
                            scr1[:], scr2[:], gtvs[:, i:i + 1]
                        )
                        nc.vector.tensor_tensor(
                            out=lg_v[:, :, ii], in0=lg_v[:, :, ii], in1=scr1[:],
                            op=OP.add,
                        )
                    dst = mneg if half == 0 else scr2
                    nc.vector.tensor_reduce(
                        out=dst[:], in_=lg_v, axis=mybir.AxisListType.X,
                        op=OP.min,
                    )
                nc.vector.tensor_tensor(
                    out=mneg[:], in0=mneg[:], in1=scr2[:], op=OP.min
                )
                tc.no_sync_barrier()
                nc.vector.tensor_tensor(
                    out=u0[:], in0=x2[:], in1=cosx[:], op=OP.mult
                )

                _strict_barrier(tc, nc)
                for dst, r, src in (
                    (stage_gf, 1, x_pm), (stage_inp, 2, t_r),
                    (stage_inp, 5, t_r), (stage_gf, 3, t_pm),
                    (stage_inp, 0, cosr), (stage_inp, 3, cosr),
                    (stage_inp, 1, sinr), (stage_inp, 4, sinr),
                    (stage_gf, 0, x2), (stage_gf, 2, t2), (stage_gf, 4, mneg),
                ):
                    d = dst[:, r, :]
                    if dst is stage_gf:
                        d = d.bitcast(F32)
                    nc.sync.dma_start(out=d, in_=src[:])

            # ---- Phase M: feature-major tile loop ----
            _strict_barrier(tc, nc)
            wps = pgo.tile([K, F], F32, tag="go")
            nc.tensor.matmul(
                out=wps[0:1, 0:2], lhsT=wpk[0:1, 0:1], rhs=wpk[0:1, 0:2],
                start=True, stop=True,
            )
            wdum2 = sm.tile([1, 4], F32, tag="wdum")
            nc.scalar.activation(wdum2[0:1, 0:1], cpk[0:1, 0:1], AF.Copy)
            nc.vector.tensor_copy(out=wdum2[0:1, 1:2], in_=cpk[0:1, 0:1])

            wslice = (w1s, w2s, w3s, w4s)
            inps = {}

            def load_inp(c):
                inp6 = ld6.tile([6, F], F16, tag="inp6", name=f"i6_{c}")
                nc.sync.dma_start(out=inp6[:], in_=stage_inp[c, :, :])
                inps[c] = inp6

            def emit_gate(c):
                """gf load + gating matmul + exp for tile c."""
                gf5 = ldg.tile([5, F], F32R, tag="gf5", name=f"g5_{c}")
                nc.sync.dma_start(out=gf5[:], in_=stage_gf[c, :, :])
                lg_ps = pgo.tile([K, F], F32, tag="go", name=f"lg{c}")
                for j in range(F // CH):
                    nc.tensor.matmul(
                        out=lg_ps[:, bass.ts(j, CH)],
                        lhsT=wpk[0:5, 0:8],
                        rhs=gf5[:, bass.ts(j, CH)],
                        start=True, stop=True,
                    )
                e_sb = ep.tile([K, F], F16, tag="e_grp", name=f"e{c}")
                nc.scalar.activation(
                    e_sb[:], lg_ps[:], AF.Exp, bias=eb4s[0:K, 0:1]
                )
                return e_sb

            def emit_l1(c):
                inp6 = inps.pop(c)
                hs = []
                for p in range(NPAIR):
                    lp = pL.tile([P, F], F32, tag="L", name=f"l1_{c}_{p}")
                    for j in range(F // CH):
                        nc.tensor.matmul(
                            out=lp[:, bass.ts(j, CH)],
                            lhsT=w1s[0:6, bass.ts(p, P)],
                            rhs=inp6[0:6, bass.ts(j, CH)],
                            start=True, stop=True,
                        )
                    hnew = hp.tile([P, F], F16, tag="h", name=f"h1_{c}_{p}")
                    nc.scalar.activation(
                        hnew[:], lp[:], AF.Tanh,
                        bias=bscs[:, p:p + 1], scale=sscs[:, p:p + 1],
                    )
                    hs.append(hnew)
                return hs

            def rational_tanh(c, p, lp, width=F):
                """DVE psum drain + 11-op fp16 rational tanh of pair p's L4
                over columns [0:width]; caller handles the rest via ACT."""
                w = width
                z = rp.tile([P, F], F16, tag="z", name=f"z{c}_{p}")
                nc.vector.tensor_scalar(
                    out=z[:, 0:w], in0=lp[:, 0:w], scalar1=ss8s[:, p:p + 1],
                    scalar2=bs8s[:, p:p + 1], op0=OP.mult, op1=OP.add,
                )
                pp = rp.tile([P, F], F16, tag="p", name=f"p{c}_{p}")
                nc.vector.tensor_tensor(
                    out=pp[:, 0:w], in0=z[:, 0:w], in1=z[:, 0:w], op=OP.mult
                )
                q = rp.tile([P, F], F16, tag="q", name=f"q{c}_{p}")
                nc.vector.tensor_scalar(
                    out=q[:, 0:w], in0=pp[:, 0:w], scalar1=RC2,
                    scalar2=RC2 * RC1, op0=OP.mult, op1=OP.add,
                )
                nc.vector.tensor_tensor(
                    out=q[:, 0:w], in0=q[:, 0:w], in1=pp[:, 0:w], op=OP.mult
                )
                nc.vector.tensor_scalar(
                    out=q[:, 0:w], in0=q[:, 0:w], scalar1=RC2 * RC0,
                    scalar2=None, op0=OP.add,
                )
                nc.vector.tensor_tensor(
                    out=q[:, 0:w], in0=q[:, 0:w], in1=z[:, 0:w], op=OP.mult
                )
                d = rp.tile([P, F], F16, tag="d", name=f"d{c}_{p}")
                nc.vector.tensor_scalar(
                    out=d[:, 0:w], in0=pp[:, 0:w], scalar1=RD1, scalar2=None,
                    op0=OP.add,
                )
                nc.vector.tensor_tensor(
                    out=d[:, 0:w], in0=d[:, 0:w], in1=pp[:, 0:w], op=OP.mult
                )
                nc.vector.tensor_scalar(
                    out=d[:, 0:w], in0=d[:, 0:w], scalar1=RD0, scalar2=None,
                    op0=OP.add,
                )
                with nc.allow_low_precision(reason="rational tanh denominator"):
                    nc.vector.reciprocal(out=d[:, 0:w], in_=d[:, 0:w])
                hnew = hp.tile([P, F], F16, tag="h", name=f"h4r_{c}_{p}")
                nc.vector.tensor_tensor(
                    out=hnew[:, 0:w], in0=q[:, 0:w], in1=d[:, 0:w], op=OP.mult
                )
                nc.vector.tensor_scalar(
                    out=hnew[:, 0:w], in0=hnew[:, 0:w], scalar1=-1.0,
                    scalar2=1.0, op0=OP.max, op1=OP.min,
                )
                return hnew

            def emit_layers(c, hs):
                """Layers 2-4 and head for tile c (L4 pairs 0/1 on DVE)."""
                for l in range(1, 4):
                    wl = wslice[l]
                    hn = []
                    for p in range(NPAIR):
                        lp = pL.tile([P, F], F32, tag="L", name=f"l{l}_{c}_{p}")
                        for j in range(F // CH):
                            nc.tensor.matmul(
                                out=lp[:, bass.ts(j, CH)],
                                lhsT=wl[:, bass.ts(p, P)],
                                rhs=hs[p][:, bass.ts(j, CH)],
                                start=True, stop=True,
                            )
                        if l == 3 and p == 0:
                            hn.append(rational_tanh(c, p, lp))
                        elif l == 3 and p == 1:
                            # split pair: DVE rational on cols 0:CH, ACT on rest
                            hnew = rational_tanh(c, p, lp, width=CH)
                            nc.scalar.activation(
                                hnew[:, CH:F], lp[:, CH:F], AF.Tanh,
                                bias=bscs[:, 4 * l + p:4 * l + p + 1],
                                scale=sscs[:, 4 * l + p:4 * l + p + 1],
                            )
                            hn.append(hnew)
                        else:
                            hnew = hp.tile(
                                [P, F], F16, tag="h", name=f"h{l}_{c}_{p}"
                            )
                            nc.scalar.activation(
                                hnew[:], lp[:], AF.Tanh,
                                bias=bscs[:, 4 * l + p:4 * l + p + 1],
                                scale=sscs[:, 4 * l + p:4 * l + p + 1],
                            )
                            hn.append(hnew)
                    hs = hn
                # head: accumulate ACT-produced pairs first, DVE pairs last
                o_ps = pgo.tile([K, F], F32, tag="go", name=f"o{c}")
                order = (2, 3, 0, 1)
                for idx, p in enumerate(order):
                    for j in range(F // CH):
                        nc.tensor.matmul(
                            out=o_ps[:, bass.ts(j, CH)],
                            lhsT=w5s[:, bass.ts(p, K)],
                            rhs=hs[p][:, bass.ts(j, CH)],
                            start=(idx == 0), stop=(idx == NPAIR - 1),
                        )
                return o_ps

            def emit_tail(c, e_sb, o_ps):
                eo = sm.tile([K, F], F16, tag="eo", name=f"eo{c}")
                nc.vector.tensor_tensor(
                    out=eo[:], in0=o_ps[:], in1=e_sb[:], op=OP.mult
                )
                place = pgo.tile([2, F], F32, tag="go", name=f"pl{c}")
                for j in range(F // CH):
                    nc.tensor.matmul(
                        out=place[:, bass.ts(j, CH)], lhsT=selB_k[0][:],
                        rhs=e_sb[:, bass.ts(j, CH)], start=True, stop=False,
                    )
                    nc.tensor.matmul(
                        out=place[0:1, bass.ts(j, CH)], lhsT=selOs[:],
                        rhs=eo[:, bass.ts(j, CH)], start=False, stop=True,
                    )
                cdsb = sm.tile([2, F], F32, tag="cdsb", name=f"cd{c}")
                nc.vector.tensor_copy(out=cdsb[:], in_=place[:])
                nc.sync.dma_start(out=comb_dram[c, :, :], in_=cdsb[:])

            # ---- prime the pipeline ----
            for c in range(2):
                load_inp(c)
            gates = {0: emit_gate(0)}
            h_cur = emit_l1(0)
            gates[1] = emit_gate(1)

            for c in range(NT):
                if c + 2 < NT:
                    load_inp(c + 2)
                o_ps = emit_layers(c, h_cur)
                if c + 2 < NT:
                    gates[c + 2] = emit_gate(c + 2)
                if c + 1 < NT:
                    h_cur = emit_l1(c + 1)
                emit_tail(c, gates.pop(c), o_ps)

            # ---- Phase F: relayout + finalize ----
            _strict_barrier(tc, nc)
            wdum3 = sm.tile([1, 4], F32, tag="wdum")
            nc.vector.tensor_copy(out=wdum3[0:1, 0:1], in_=cpk[0:1, 0:1])
            dND = pm.tile([P, 2 * NJ], F32, tag="dND")
            nc.sync.dma_start(
                out=dND[:], in_=comb_dram[:].rearrange("p r j -> p (r j)")
            )
            dN = dND[:, 0:NJ]
            dD = dND[:, NJ:2 * NJ]
            dinv = pm.tile([P, NJ], F32, tag="dinv")
            res = pm.tile([P, NJ], F32, tag="res")
            nc.vector.reciprocal(dinv[:], dD)
            nc.vector.tensor_tensor(out=res[:], in0=dN, in1=dinv[:], op=OP.mult)
            nc.vector.tensor_tensor(out=res[:], in0=res[:], in1=tanht[:], op=OP.mult)
            nc.vector.tensor_tensor(out=res[:], in0=res[:], in1=u0[:], op=OP.add)
            nc.sync.dma_start(out=u_out[:], in_=res[:])

    nc.compile()
    return nc


def _prep_host(inputs):
    """Build the derived parameter arrays (tiny, replicated across cores)."""
    W1, b1 = inputs["W1"], inputs["b1"]      # [K,H,3], [K,H]
    W2, b2 = inputs["W2"], inputs["b2"]
    W3, b3 = inputs["W3"], inputs["b3"]
    W4, b4 = inputs["W4"], inputs["b4"]
    W5, b5 = inputs["W5"], inputs["b5"]      # [K,1,H], [K,1]
    scales = inputs["scales"]                # [K,4]
    centers = inputs["centers"]              # [K,2]
    log_gammas = inputs["log_gammas"]        # [K,2]

    f32 = np.float32
    w1l = np.zeros((6, NPAIR * P), f32)
    w2l = np.zeros((P, NPAIR * P), f32)
    w3l = np.zeros((P, NPAIR * P), f32)
    w4l = np.zeros((P, NPAIR * P), f32)
    w5l = np.zeros((P, NPAIR * K), f32)
    for p in range(NPAIR):
        a, b = 2 * p, 2 * p + 1
        w1l[0:3, p * P:p * P + H] = W1[a].T
        w1l[3:6, p * P + H:(p + 1) * P] = W1[b].T
        for wl, Wsrc in ((w2l, W2), (w3l, W3), (w4l, W4)):
            wl[0:H, p * P:p * P + H] = Wsrc[a].T
            wl[H:P, p * P + H:(p + 1) * P] = Wsrc[b].T
        w5l[0:H, p * K + a] = W5[a][0]
        w5l[H:P, p * K + b] = W5[b][0]

    bsc = np.zeros((P, 16), f32)
    ssc = np.zeros((P, 16), f32)
    blist = (b1, b2, b3, b4)
    for l in range(4):
        for p in range(NPAIR):
            a, b = 2 * p, 2 * p + 1
            col = 4 * l + p
            bsc[0:H, col] = scales[a, l] * blist[l][a]
            bsc[H:P, col] = scales[b, l] * blist[l][b]
            ssc[0:H, col] = scales[a, l]
            ssc[H:P, col] = scales[b, l]

    selB = np.zeros((K, 2), f32)
    selO = np.ones((K, 1), f32)
    selB[:, 0] = b5[:, 0]
    selB[:, 1] = 1.0

    gam = np.exp(log_gammas).astype(np.float64)
    cx, ct = centers[:, 0].astype(np.float64), centers[:, 1].astype(np.float64)
    gx, gt = gam[:, 0], gam[:, 1]
    g5m = np.zeros((5, K), f32)
    g5m[0] = -gx
    g5m[1] = 2.0 * gx * cx
    g5m[2] = -gt
    g5m[3] = 2.0 * gt * ct
    g5m[4] = 1.0
    ebias = (-(gx * cx * cx + gt * ct * ct)).astype(f32).reshape(K, 1)

    cxn = np.tile((-cx).astype(f32), (P, 1))
    ctn = np.tile((-ct).astype(f32), (P, 1))
    gxv = np.tile(gx.astype(f32), (P, 1))
    gtv = np.tile(gt.astype(f32), (P, 1))

    trigc = np.zeros((P, 2), f32)
    trigc[:, 0] = np.pi / 2

    cpack = np.zeros((P, CPACK_W), f32)
    wpack = np.zeros((P, WPACK_W), f32)
    wpk16 = np.zeros((P, W16_W), np.float16)

    wpack[0:5, 0:8] = g5m

    def put16(name, arr):
        h, w = arr.shape
        wpk16[0:h, W16[name]:W16[name] + w] = arr.astype(np.float16)

    put16("w1", w1l)
    put16("w2", w2l)
    put16("w3", w3l)
    put16("w4", w4l)
    put16("w5", w5l)
    for k in range(3):
        wpk16[32 * k:32 * k + K, W16["selB"]:W16["selB"] + 2] = (
            selB.astype(np.float16))
    put16("selO", selO)

    def put(name, arr):
        h, w = arr.shape
        cpack[0:h, COL[name]:COL[name] + w] = arr

    put("bsc", bsc)
    put("ssc", ssc)
    put("bs8", bsc[:, 12:14] / 8.0)
    put("ss8", ssc[:, 12:14] / 8.0)
    eb72 = np.zeros((72, 1), f32)
    for k in range(3):
        eb72[32 * k:32 * k + 8] = ebias
    put("eb4", eb72)
    put("cxn", cxn)
    put("ctn", ctn)
    put("gxv", gxv)
    put("gtv", gtv)
    put("trig", trigc)
    return dict(cpack=cpack, wpack=wpack, wpk16=wpk16)


def kernel(**inputs):
    inputs = {k: np.asarray(v) for k, v in inputs.items()}
    x = inputs["x"].astype(np.float32).reshape(N)
    t = inputs["t"].astype(np.float32).reshape(N)

    if "nc" not in _CACHE:
        _CACHE["nc"] = _build_program()
    nc = _CACHE["nc"]

    params = _prep_host(inputs)
    in_maps = []
    for i in range(NCORES):
        sl = slice(i * NC_PTS, (i + 1) * NC_PTS)
        m = dict(params)
        m["x_in"] = np.ascontiguousarray(x[sl].reshape(P, NJ))
        m["t_in"] = np.ascontiguousarray(t[sl].reshape(P, NJ))
        in_maps.append(m)

    res = run_bass_kernel_spmd(nc, in_maps, list(range(NCORES)))
    out = np.empty((N,), np.float32)
    for i in range(NCORES):
        out[i * NC_PTS:(i + 1) * NC_PTS] = res.results[i]["u_out"].reshape(NC_PTS)
    return out.reshape(N, 1)


if __name__ == "__main__":
    print("smoke test: building program")
    _build_program()
    print("ok")


# revision 13
# speedup vs baseline: 1.2977x; 1.2977x over previous
"""Trainium2 Bass kernel for DynamicABPINN (moe_routing, dense evaluation).

Model: 8 gated subnets (4 hidden tanh layers of width 64 each), Gaussian-window
softmax gating over subnets, periodic input embedding, hard-constraint output.

Strategy (v2, ACT-bottleneck optimized):
  - Pure data parallel over 8 NeuronCores: each core handles N/8 = 131072 points.
  - All matmul datapaths fp16/f32r (1 cyc/row); hidden state fp16 end to end.
  - Gating exp batched across groups of 4 tiles into one [32, F] psum via
    sparse-column lhsT accumulation; one ACT Exp per group.
  - ACT does 14 of 16 pair-layer tanhs per tile; layer-4 pairs 0/1 are
    offloaded: GPSIMD (Pool) drains psum -> fp16 with scale/8,bias/8, then an
    11-op DVE fp16 rational approximates tanh (max err ~1.5e-3, end-to-end
    rel err ~8e-4 incl. fp16 rounding).
  - Combine: eo on DVE, numer/denom via one fp16 place matmul pair into psum
    partitions 32:34 (tile_position col 32), Pool drains to SBUF, DMA out.
"""

import sys

for _p in ("/opt/trn_rl_repo", "/root/.axon_site/_ro/trn_rl_repo"):
    if _p not in sys.path:
        sys.path.insert(0, _p)

import numpy as np

import concourse.bass as bass
import concourse.bacc as bacc
import concourse.mybir as mybir
from concourse.tile import TileContext
from concourse.tile_rust import add_dep_helper
from concourse.bass_utils import run_bass_kernel_spmd


def _strict_barrier(tc, nc):
    """strict_bb_all_engine_barrier anchored on a DRAIN instruction (walrus
    caps queue instructions at one embedded sem wait, except DRAIN)."""
    curr_bb = nc.cur_bb
    assert curr_bb is not None
    prev_insts = list(curr_bb.bb.instructions)
    barrier_instruction = nc.sync.drain()
    tc.barrier_instruction_and_bb = (barrier_instruction.ins, curr_bb)
    if (
        tc.no_sync_barrier_and_bb is not None
        and tc.no_sync_barrier_and_bb[1] == curr_bb
    ):
        tc.no_sync_barrier_and_bb = None
    for instruction in prev_insts:
        add_dep_helper(
            barrier_instruction.ins,
            instruction,
            sync=bass.sync_unless_reorderable_target(
                instruction, instruction.is_executable()
            ),
            reason="strict_bb_all_engine_barrier: backward edge",
        )

F32 = mybir.dt.float32
F32R = mybir.dt.float32r
F16 = mybir.dt.float16
AF = mybir.ActivationFunctionType
OP = mybir.AluOpType

N = 1048576
NCORES = 8
NC_PTS = N // NCORES          # 131072 points per core
P = 128                       # partitions
NJ = NC_PTS // P              # 1024 point-major columns
NT = P                        # 128 feature-major tiles of F=1024 points
F = NJ                        # 1024 points per tile
CH = 512                      # matmul moving-operand chunk (fp32 max, 1 psum bank)
K = 8                         # subnets
H = 64                        # hidden width
NPAIR = 4                     # subnet pairs packed into 128 partitions
PI = float(np.pi)

# rational tanh(y) ~ clamp(C2*z*(p^2+c1'p+c0')/(p^2+d1'p+d0')), z=y/8, p=z^2
_RC0, _RC1, _RC2 = 1.45060304e+03, 1.49308936e+02, 5.30585751e-02
_RD0, _RD1 = 7.69755550e+01, 3.35529749e+01
RC1 = _RC1 / 64.0
RC0 = _RC0 / 4096.0
RD1 = _RD1 / 64.0
RD0 = _RD0 / 4096.0
RC2 = 8.0 * _RC2

# column offsets inside the packed fp32 constant tensor
_COL_SIZES = (
    ("bsc", 16), ("ssc", 16), ("bs8", 2), ("ss8", 2), ("eb4", 1),
    ("cxn", K), ("ctn", K), ("gxv", K), ("gtv", K), ("trig", 2),
)
COL = {}
_off = 0
for _name, _sz in _COL_SIZES:
    COL[_name] = _off
    _off += _sz
CPACK_W = _off

# float32r pack: 4 slot-variant gating lhsT blocks [5, 32] each
WPACK_W = 128

# fp16 weight pack
_W16_SIZES = (
    ("w1", NPAIR * P), ("w2", NPAIR * P), ("w3", NPAIR * P), ("w4", NPAIR * P),
    ("w5", NPAIR * K), ("selB", 2), ("selO", 1),
)
W16 = {}
_off = 0
for _name, _sz in _W16_SIZES:
    W16[_name] = _off
    _off += _sz
W16_W = _off

_CACHE = {}


def _build_program():
    nc = bacc.Bacc()

    x_in = nc.declare_dram_parameter("x_in", [P, NJ], F32, isOutput=False)
    t_in = nc.declare_dram_parameter("t_in", [P, NJ], F32, isOutput=False)
    cpack = nc.declare_dram_parameter("cpack", [P, CPACK_W], F32, isOutput=False)
    wpack = nc.declare_dram_parameter(
        "wpack", [P, WPACK_W], F32R, isOutput=False
    )
    wpk16 = nc.declare_dram_parameter("wpk16", [P, W16_W], F16, isOutput=False)
    u_out = nc.declare_dram_parameter("u_out", [P, NJ], F32, isOutput=True)

    # Internal DRAM staging: point-major -> feature-major relayout.
    stage_inp = nc.dram_tensor("stage_inp", [NT, 6, F], F16)
    stage_gf = nc.dram_tensor("stage_gf", [NT, 5, F], F32R)
    comb_dram = nc.dram_tensor("comb_dram", [NT, 2, F], F32)

    with TileContext(nc) as tc:
        with (
            tc.tile_pool(name="const", bufs=1) as cpool,
            tc.tile_pool(name="pm", bufs=1) as pm,
            tc.tile_pool(name="ld6", bufs=3) as ld6,
            tc.tile_pool(name="ldg", bufs=3) as ldg,
            tc.tile_pool(name="hpool", bufs=12) as hp,
            tc.tile_pool(name="small", bufs=2) as sm,
            tc.tile_pool(name="epool", bufs=3) as ep,
            tc.tile_pool(name="rat", bufs=2) as rp,
            tc.tile_pool(name="pL", bufs=2, space="PSUM") as pL,
            tc.tile_pool(name="pgo", bufs=2, space="PSUM") as pgo,
        ):
            # ---- constants ----
            cpk = cpool.tile([P, CPACK_W], F32, tag="cpk")
            nc.sync.dma_start(out=cpk[:], in_=cpack[:])
            wpk = cpool.tile([P, WPACK_W], F32R, tag="wpk")
            nc.sync.dma_start(out=wpk[:], in_=wpack[:])
            wk16 = cpool.tile([P, W16_W], F16, tag="wk16")
            nc.sync.dma_start(out=wk16[:], in_=wpk16[:])

            w1s = wk16[0:6, W16["w1"]:W16["w1"] + NPAIR * P]
            w2s = wk16[:, W16["w2"]:W16["w2"] + NPAIR * P]
            w3s = wk16[:, W16["w3"]:W16["w3"] + NPAIR * P]
            w4s = wk16[:, W16["w4"]:W16["w4"] + NPAIR * P]
            w5s = wk16[:, W16["w5"]:W16["w5"] + NPAIR * K]
            selB_k = [wk16[32 * k:32 * k + K, W16["selB"]:W16["selB"] + 2]
                      for k in range(3)]
            selOs = wk16[0:K, W16["selO"]:W16["selO"] + 1]
            bscs = cpk[:, COL["bsc"]:COL["bsc"] + 16]
            sscs = cpk[:, COL["ssc"]:COL["ssc"] + 16]
            bs8s = cpk[:, COL["bs8"]:COL["bs8"] + 2]
            ss8s = cpk[:, COL["ss8"]:COL["ss8"] + 2]
            eb4s = cpk[0:72, COL["eb4"]:COL["eb4"] + 1]
            cxns = cpk[:, COL["cxn"]:COL["cxn"] + K]
            ctns = cpk[:, COL["ctn"]:COL["ctn"] + K]
            gxvs = cpk[:, COL["gxv"]:COL["gxv"] + K]
            gtvs = cpk[:, COL["gtv"]:COL["gtv"] + K]
            trgc = cpk[:, COL["trig"]:COL["trig"] + 2]

            # engine clock warmup (single embedded sem wait per engine)
            wdum = sm.tile([1, 4], F32, tag="wdum")
            nc.scalar.activation(wdum[0:1, 0:1], cpk[0:1, 0:1], AF.Copy)

            # ---- Phase S: point-major prep ----
            tanht = pm.tile([P, NJ], F32, tag="tanht")
            u0 = pm.tile([P, NJ], F32, tag="u0")
            with tc.tile_pool(name="sphase", bufs=1) as sp:
                x_pm = sp.tile([P, NJ], F32, tag="x_pm")
                t_pm = sp.tile([P, NJ], F32, tag="t_pm")
                nc.sync.dma_start(out=x_pm[:], in_=x_in[:])
                nc.sync.dma_start(out=t_pm[:], in_=t_in[:])

                cosx = sp.tile([P, NJ], F32, tag="cosx")
                cosr = sp.tile([P, NJ], F16, tag="cosr")
                sinr = sp.tile([P, NJ], F16, tag="sinr")
                t_r = sp.tile([P, NJ], F16, tag="t_r")
                x2 = sp.tile([P, NJ], F32, tag="x2")
                t2 = sp.tile([P, NJ], F32, tag="t2")
                mneg = sp.tile([P, NJ], F32, tag="mneg")
                lg_all = sp.tile([P, NJ * (K // 2)], F32, tag="lg_all")

                # ACT Sin has no range reduction: reduce args to [-1, 1]
                # periods via fp32 magic-number rounding, r = z - 2*round(z/2).
                MAGIC = float(1.5 * 2 ** 23)
                scr1 = sp.tile([P, NJ], F32, tag="scr1")
                scr2 = sp.tile([P, NJ], F32, tag="scr2")
                # sin(pi x): r = x - 2 round(x/2)
                nc.vector.tensor_scalar(
                    out=scr1[:], in0=x_pm[:], scalar1=0.5, scalar2=MAGIC,
                    op0=OP.mult, op1=OP.add,
                )
                nc.vector.tensor_scalar(
                    out=scr1[:], in0=scr1[:], scalar1=MAGIC, scalar2=-2.0,
                    op0=OP.subtract, op1=OP.mult,
                )
                nc.vector.tensor_tensor(
                    out=scr1[:], in0=x_pm[:], in1=scr1[:], op=OP.add
                )
                nc.scalar.activation(
                    sinr[:], scr1[:], AF.Sin, bias=trgc[:, 1:2], scale=PI
                )
                # cos(pi x) = sin(pi (x + 1/2)): same reduction on y = x + 0.5
                nc.vector.tensor_scalar(
                    out=scr2[:], in0=x_pm[:], scalar1=0.5, scalar2=0.25,
                    op0=OP.mult, op1=OP.add,
                )
                nc.vector.tensor_scalar(
                    out=scr2[:], in0=scr2[:], scalar1=MAGIC, scalar2=MAGIC,
                    op0=OP.add, op1=OP.subtract,
                )
                nc.vector.tensor_scalar(
                    out=scr2[:], in0=scr2[:], scalar1=-2.0, scalar2=0.5,
                    op0=OP.mult, op1=OP.add,
                )
                nc.vector.tensor_tensor(
                    out=scr2[:], in0=x_pm[:], in1=scr2[:], op=OP.add
                )
                nc.scalar.activation(
                    cosx[:], scr2[:], AF.Sin, bias=trgc[:, 1:2], scale=PI
                )
                nc.scalar.activation(
                    cosr[:], scr2[:], AF.Sin, bias=trgc[:, 1:2], scale=PI
                )
                nc.scalar.activation(t_r[:], t_pm[:], AF.Copy)
                nc.scalar.activation(tanht[:], t_pm[:], AF.Tanh, bias=trgc[:, 1:2])
                nc.vector.tensor_tensor(
                    out=x2[:], in0=x_pm[:], in1=x_pm[:], op=OP.mult
                )
                nc.vector.tensor_tensor(
                    out=t2[:], in0=t_pm[:], in1=t_pm[:], op=OP.mult
                )
                tc.no_sync_barrier()

                # gating quadratics, interleaved [P, NJ, K/2] x 2 halves
                lg_v = lg_all[:].rearrange("p (j k) -> p j k", k=K // 2)
                for half in range(2):
                    for ii in range(K // 2):
                        i = half * (K // 2) + ii
                        nc.vector.tensor_scalar_add(
                            scr1[:], x_pm[:], cxns[:, i:i + 1]
                        )
                        nc.vector.tensor_tensor(
                            out=scr2[:], in0=scr1[:], in1=scr1[:], op=OP.mult
                        )
                        nc.vector.tensor_scalar_mul(
                            lg_v[:, :, ii], scr2[:], gxvs[:, i:i + 1]
                        )
                        nc.vector.tensor_scalar_add(
                            scr1[:], t_pm[:], ctns[:, i:i + 1]
                        )
                        nc.vector.tensor_tensor(
                            out=scr2[:], in0=scr1[:], in1=scr1[:], op=OP.mult
                        )
                        nc.vector.tensor_scalar_mul(
                            scr1[:], scr2[:], gtvs[:, i:i + 1]
                        )
                        nc.vector.tensor_tensor(
                            out=lg_v[:, :, ii], in0=lg_v[:, :, ii], in1=scr1[:],
                            op=OP.add,
                        )
                    dst = mneg if half == 0 else scr2
                    nc.vector.tensor_reduce(
                        out=dst[:], in_=lg_v, axis=mybir.AxisListType.X,
                        op=OP.min,
                    )
                nc.vector.tensor_tensor(
                    out=mneg[:], in0=mneg[:], in1=scr2[:], op=OP.min
                )
                tc.no_sync_barrier()
                nc.vector.tensor_tensor(
                    out=u0[:], in0=x2[:], in1=cosx[:], op=OP.mult
                )

                _strict_barrier(tc, nc)
                for dst, r, src in (
                    (stage_gf, 1, x_pm), (stage_inp, 2, t_r),
                    (stage_inp, 5, t_r), (stage_gf, 3, t_pm),
                    (stage_inp, 0, cosr), (stage_inp, 3, cosr),
                    (stage_inp, 1, sinr), (stage_inp, 4, sinr),
                    (stage_gf, 0, x2), (stage_gf, 2, t2), (stage_gf, 4, mneg),
                ):
                    d = dst[:, r, :]
                    if dst is stage_gf:
                        d = d.bitcast(F32)
                    nc.sync.dma_start(out=d, in_=src[:])

            # ---- Phase M: feature-major tile loop ----
            _strict_barrier(tc, nc)
            wps = pgo.tile([K, F], F32, tag="go")
            nc.tensor.matmul(
                out=wps[0:1, 0:2], lhsT=wpk[0:1, 0:1], rhs=wpk[0:1, 0:2],
                start=True, stop=True,
            )
            wdum2 = sm.tile([1, 4], F32, tag="wdum")
            nc.scalar.activation(wdum2[0:1, 0:1], cpk[0:1, 0:1], AF.Copy)
            nc.vector.tensor_copy(out=wdum2[0:1, 1:2], in_=cpk[0:1, 0:1])

            wslice = (w1s, w2s, w3s, w4s)
            inps = {}

            def load_inp(c):
                inp6 = ld6.tile([6, F], F16, tag="inp6", name=f"i6_{c}")
                nc.sync.dma_start(out=inp6[:], in_=stage_inp[c, :, :])
                inps[c] = inp6

            def emit_gate(c):
                """gf load + gating matmul + exp for tile c."""
                gf5 = ldg.tile([5, F], F32R, tag="gf5", name=f"g5_{c}")
                nc.sync.dma_start(out=gf5[:], in_=stage_gf[c, :, :])
                lg_ps = pgo.tile([K, F], F32, tag="go", name=f"lg{c}")
                for j in range(F // CH):
                    nc.tensor.matmul(
                        out=lg_ps[:, bass.ts(j, CH)],
                        lhsT=wpk[0:5, 0:8],
                        rhs=gf5[:, bass.ts(j, CH)],
                        start=True, stop=True,
                    )
                e_sb = ep.tile([K, F], F16, tag="e_grp", name=f"e{c}")
                nc.scalar.activation(
                    e_sb[:], lg_ps[:], AF.Exp, bias=eb4s[0:K, 0:1]
                )
                return e_sb

            def emit_l1(c):
                inp6 = inps.pop(c)
                hs = []
                for p in range(NPAIR):
                    lp = pL.tile([P, F], F32, tag="L", name=f"l1_{c}_{p}")
                    for j in range(F // CH):
                        nc.tensor.matmul(
                            out=lp[:, bass.ts(j, CH)],
                            lhsT=w1s[0:6, bass.ts(p, P)],
                            rhs=inp6[0:6, bass.ts(j, CH)],
                            start=True, stop=True,
                        )
                    hnew = hp.tile([P, F], F16, tag="h", name=f"h1_{c}_{p}")
                    nc.scalar.activation(
                        hnew[:], lp[:], AF.Tanh,
                        bias=bscs[:, p:p + 1], scale=sscs[:, p:p + 1],
                    )
                    hs.append(hnew)
                return hs

            def rational_tanh(c, p, lp, width=F):
                """DVE psum drain + 11-op fp16 rational tanh of pair p's L4
                over columns [0:width]; caller handles the rest via ACT."""
                w = width
                z = rp.tile([P, F], F16, tag="z", name=f"z{c}_{p}")
                nc.vector.tensor_scalar(
                    out=z[:, 0:w], in0=lp[:, 0:w], scalar1=ss8s[:, p:p + 1],
                    scalar2=bs8s[:, p:p + 1], op0=OP.mult, op1=OP.add,
                )
                pp = rp.tile([P, F], F16, tag="p", name=f"p{c}_{p}")
                nc.vector.tensor_tensor(
                    out=pp[:, 0:w], in0=z[:, 0:w], in1=z[:, 0:w], op=OP.mult
                )
                q = rp.tile([P, F], F16, tag="q", name=f"q{c}_{p}")
                nc.vector.tensor_scalar(
                    out=q[:, 0:w], in0=pp[:, 0:w], scalar1=RC2,
                    scalar2=RC2 * RC1, op0=OP.mult, op1=OP.add,
                )
                nc.vector.tensor_tensor(
                    out=q[:, 0:w], in0=q[:, 0:w], in1=pp[:, 0:w], op=OP.mult
                )
                nc.vector.tensor_scalar(
                    out=q[:, 0:w], in0=q[:, 0:w], scalar1=RC2 * RC0,
                    scalar2=None, op0=OP.add,
                )
                nc.vector.tensor_tensor(
                    out=q[:, 0:w], in0=q[:, 0:w], in1=z[:, 0:w], op=OP.mult
                )
                d = rp.tile([P, F], F16, tag="d", name=f"d{c}_{p}")
                nc.vector.tensor_scalar(
                    out=d[:, 0:w], in0=pp[:, 0:w], scalar1=RD1, scalar2=None,
                    op0=OP.add,
                )
                nc.vector.tensor_tensor(
                    out=d[:, 0:w], in0=d[:, 0:w], in1=pp[:, 0:w], op=OP.mult
                )
                nc.vector.tensor_scalar(
                    out=d[:, 0:w], in0=d[:, 0:w], scalar1=RD0, scalar2=None,
                    op0=OP.add,
                )
                with nc.allow_low_precision(reason="rational tanh denominator"):
                    nc.vector.reciprocal(out=d[:, 0:w], in_=d[:, 0:w])
                hnew = hp.tile([P, F], F16, tag="h", name=f"h4r_{c}_{p}")
                nc.vector.tensor_tensor(
                    out=hnew[:, 0:w], in0=q[:, 0:w], in1=d[:, 0:w], op=OP.mult
                )
                nc.vector.tensor_scalar(
                    out=hnew[:, 0:w], in0=hnew[:, 0:w], scalar1=-1.0,
                    scalar2=1.0, op0=OP.max, op1=OP.min,
                )
                return hnew

            def emit_layers(c, hs):
                """Layers 2-4 and head for tile c (L4 pairs 0/1 on DVE)."""
                for l in range(1, 4):
                    wl = wslice[l]
                    hn = []
                    for p in range(NPAIR):
                        lp = pL.tile([P, F], F32, tag="L", name=f"l{l}_{c}_{p}")
                        for j in range(F // CH):
                            nc.tensor.matmul(
                                out=lp[:, bass.ts(j, CH)],
                                lhsT=wl[:, bass.ts(p, P)],
                                rhs=hs[p][:, bass.ts(j, CH)],
                                start=True, stop=True,
                            )
                        if False:
                            hn.append(rational_tanh(c, p, lp))
                        else:
                            hnew = hp.tile(
                                [P, F], F16, tag="h", name=f"h{l}_{c}_{p}"
                            )
                            nc.scalar.activation(
                                hnew[:], lp[:], AF.Tanh,
                                bias=bscs[:, 4 * l + p:4 * l + p + 1],
                                scale=sscs[:, 4 * l + p:4 * l + p + 1],
                            )
                            hn.append(hnew)
                    hs = hn
                # head: accumulate ACT-produced pairs first, DVE pairs last
                o_ps = pgo.tile([K, F], F32, tag="go", name=f"o{c}")
                order = (2, 3, 0, 1)
                for idx, p in enumerate(order):
                    for j in range(F // CH):
                        nc.tensor.matmul(
                            out=o_ps[:, bass.ts(j, CH)],
                            lhsT=w5s[:, bass.ts(p, K)],
                            rhs=hs[p][:, bass.ts(j, CH)],
                            start=(idx == 0), stop=(idx == NPAIR - 1),
                        )
                return o_ps

            def emit_tail(c, e_sb, o_ps):
                eo = sm.tile([K, F], F16, tag="eo", name=f"eo{c}")
                nc.vector.tensor_tensor(
                    out=eo[:], in0=o_ps[:], in1=e_sb[:], op=OP.mult
                )
                place = pgo.tile([2, F], F32, tag="go", name=f"pl{c}")
                for j in range(F // CH):
                    nc.tensor.matmul(
                        out=place[:, bass.ts(j, CH)], lhsT=selB_k[0][:],
                        rhs=e_sb[:, bass.ts(j, CH)], start=True, stop=False,
                    )
                    nc.tensor.matmul(
                        out=place[0:1, bass.ts(j, CH)], lhsT=selOs[:],
                        rhs=eo[:, bass.ts(j, CH)], start=False, stop=True,
                    )
                cdsb = sm.tile([2, F], F32, tag="cdsb", name=f"cd{c}")
                nc.vector.tensor_copy(out=cdsb[:], in_=place[:])
                nc.sync.dma_start(out=comb_dram[c, :, :], in_=cdsb[:])

            # ---- prime the pipeline ----
            for c in range(2):
                load_inp(c)
            gates = {0: emit_gate(0)}
            h_cur = emit_l1(0)
            gates[1] = emit_gate(1)

            for c in range(NT):
                if c + 2 < NT:
                    load_inp(c + 2)
                o_ps = emit_layers(c, h_cur)
                if c + 2 < NT:
                    gates[c + 2] = emit_gate(c + 2)
                if c + 1 < NT:
                    h_cur = emit_l1(c + 1)
                emit_tail(c, gates.pop(c), o_ps)

            # ---- Phase F: relayout + finalize ----
            _strict_barrier(tc, nc)
            wdum3 = sm.tile([1, 4], F32, tag="wdum")
            nc.vector.tensor_copy(out=wdum3[0:1, 0:1], in_=cpk[0:1, 0:1])
            dND = pm.tile([P, 2 * NJ], F32, tag="dND")
            nc.sync.dma_start(
                out=dND[:], in_=comb_dram[:].rearrange("p r j -> p (r j)")
            )
            dN = dND[:, 0:NJ]
            dD = dND[:, NJ:2 * NJ]
            dinv = pm.tile([P, NJ], F32, tag="dinv")
            res = pm.tile([P, NJ], F32, tag="res")
            nc.vector.reciprocal(dinv[:], dD)
            nc.vector.tensor_tensor(out=res[:], in0=dN, in1=dinv[:], op=OP.mult)
            nc.vector.tensor_tensor(out=res[:], in0=res[:], in1=tanht[:], op=OP.mult)
            nc.vector.tensor_tensor(out=res[:], in0=res[:], in1=u0[:], op=OP.add)
            nc.sync.dma_start(out=u_out[:], in_=res[:])

    nc.compile()
    return nc


def _prep_host(inputs):
    """Build the derived parameter arrays (tiny, replicated across cores)."""
    W1, b1 = inputs["W1"], inputs["b1"]      # [K,H,3], [K,H]
    W2, b2 = inputs["W2"], inputs["b2"]
    W3, b3 = inputs["W3"], inputs["b3"]
    W4, b4 = inputs["W4"], inputs["b4"]
    W5, b5 = inputs["W5"], inputs["b5"]      # [K,1,H], [K,1]
    scales = inputs["scales"]                # [K,4]
    centers = inputs["centers"]              # [K,2]
    log_gammas = inputs["log_gammas"]        # [K,2]

    f32 = np.float32
    w1l = np.zeros((6, NPAIR * P), f32)
    w2l = np.zeros((P, NPAIR * P), f32)
    w3l = np.zeros((P, NPAIR * P), f32)
    w4l = np.zeros((P, NPAIR * P), f32)
    w5l = np.zeros((P, NPAIR * K), f32)
    for p in range(NPAIR):
        a, b = 2 * p, 2 * p + 1
        w1l[0:3, p * P:p * P + H] = W1[a].T
        w1l[3:6, p * P + H:(p + 1) * P] = W1[b].T
        for wl, Wsrc in ((w2l, W2), (w3l, W3), (w4l, W4)):
            wl[0:H, p * P:p * P + H] = Wsrc[a].T
            wl[H:P, p * P + H:(p + 1) * P] = Wsrc[b].T
        w5l[0:H, p * K + a] = W5[a][0]
        w5l[H:P, p * K + b] = W5[b][0]

    bsc = np.zeros((P, 16), f32)
    ssc = np.zeros((P, 16), f32)
    blist = (b1, b2, b3, b4)
    for l in range(4):
        for p in range(NPAIR):
            a, b = 2 * p, 2 * p + 1
            col = 4 * l + p
            bsc[0:H, col] = scales[a, l] * blist[l][a]
            bsc[H:P, col] = scales[b, l] * blist[l][b]
            ssc[0:H, col] = scales[a, l]
            ssc[H:P, col] = scales[b, l]

    selB = np.zeros((K, 2), f32)
    selO = np.ones((K, 1), f32)
    selB[:, 0] = b5[:, 0]
    selB[:, 1] = 1.0

    gam = np.exp(log_gammas).astype(np.float64)
    cx, ct = centers[:, 0].astype(np.float64), centers[:, 1].astype(np.float64)
    gx, gt = gam[:, 0], gam[:, 1]
    g5m = np.zeros((5, K), f32)
    g5m[0] = -gx
    g5m[1] = 2.0 * gx * cx
    g5m[2] = -gt
    g5m[3] = 2.0 * gt * ct
    g5m[4] = 1.0
    ebias = (-(gx * cx * cx + gt * ct * ct)).astype(f32).reshape(K, 1)

    cxn = np.tile((-cx).astype(f32), (P, 1))
    ctn = np.tile((-ct).astype(f32), (P, 1))
    gxv = np.tile(gx.astype(f32), (P, 1))
    gtv = np.tile(gt.astype(f32), (P, 1))

    trigc = np.zeros((P, 2), f32)
    trigc[:, 0] = np.pi / 2

    cpack = np.zeros((P, CPACK_W), f32)
    wpack = np.zeros((P, WPACK_W), f32)
    wpk16 = np.zeros((P, W16_W), np.float16)

    wpack[0:5, 0:8] = g5m

    def put16(name, arr):
        h, w = arr.shape
        wpk16[0:h, W16[name]:W16[name] + w] = arr.astype(np.float16)

    put16("w1", w1l)
    put16("w2", w2l)
    put16("w3", w3l)
    put16("w4", w4l)
    put16("w5", w5l)
    for k in range(3):
        wpk16[32 * k:32 * k + K, W16["selB"]:W16["selB"] + 2] = (
            selB.astype(np.float16))
    put16("selO", selO)

    def put(name, arr):
        h, w = arr.shape
        cpack[0:h, COL[name]:COL[name] + w] = arr

    put("bsc", bsc)
    put("ssc", ssc)
    put("bs8", bsc[:, 12:14] / 8.0)
    put("ss8", ssc[:, 12:14] / 8.0)
    eb72 = np.zeros((72, 1), f32)
    for k in range(3):
        eb72[32 * k:32 * k + 8] = ebias
    put("eb4", eb72)
    put("cxn", cxn)
    put("ctn", ctn)
    put("gxv", gxv)
    put("gtv", gtv)
    put("trig", trigc)
    return dict(cpack=cpack, wpack=wpack, wpk16=wpk16)


def kernel(**inputs):
    inputs = {k: np.asarray(v) for k, v in inputs.items()}
    x = inputs["x"].astype(np.float32).reshape(N)
    t = inputs["t"].astype(np.float32).reshape(N)

    if "nc" not in _CACHE:
        _CACHE["nc"] = _build_program()
    nc = _CACHE["nc"]

    params = _prep_host(inputs)
    in_maps = []
    for i in range(NCORES):
        sl = slice(i * NC_PTS, (i + 1) * NC_PTS)
        m = dict(params)
        m["x_in"] = np.ascontiguousarray(x[sl].reshape(P, NJ))
        m["t_in"] = np.ascontiguousarray(t[sl].reshape(P, NJ))
        in_maps.append(m)

    res = run_bass_kernel_spmd(nc, in_maps, list(range(NCORES)))
    out = np.empty((N,), np.float32)
    for i in range(NCORES):
        out[i * NC_PTS:(i + 1) * NC_PTS] = res.results[i]["u_out"].reshape(NC_PTS)
    return out.reshape(N, 1)


if __name__ == "__main__":
    print("smoke test: building program")
    _build_program()
    print("ok")
